# revision 17
# baseline (speedup 1.0000x reference)
"""Trainium2 Bass kernel for nn_Attention (LN -> QKV -> softmax attn -> out proj).

Sharding: 8 cores; core c handles batch b=c//4 and heads [4*(c%4), 4*(c%4)+4).
Each core emits one bf16 partial [DIM, T]; the host sums 4 partials per batch,
transposes, and adds b_out.

Device pipeline per core (fp16 matmuls, fp32 PSUM):
  A) One pass over x^T tiles with x as the PE stationary:
     v = x^T-tile @ wv_aug (wv with a ones rider column -> Sx lands in column
     layout for free); Sxx via tiny N=1 matmuls with Square(x) stationary.
     rsqrt(var+eps) via bit-trick + Newton on the DVE, all in SBUF column
     layout (no DMA transposes). LN mean/gamma folds applied algebraically:
     qk via rank-1 (-mu u) PSUM accumulation, v via DVE outer-product fixup.
  B) QK projection: stationary = folded weights, moving = x^T -> q,k
     transposed [dh, t] with both heads of a stack split across partition
     halves. r_q folded into q via broadcast multiply; r_k rides the exp's
     per-partition scale.
  C) Attention in head pairs (one stack at a time): the two heads' score
     matmuls run concurrently in disjoint PE row groups (K=64 row tiling);
     one 2048-wide Exp per (pair, qp, kt) covers both heads; optionally a
     512-chunk is offloaded to the DVE as a bit-trick fast exp. P@V with a
     ones column appended to v accumulates the denominator in the same PSUM
     tile; per-qp reciprocal via DVE HW divide, normalize fused on copy-out.
  D) Output projection: both stacks accumulated into one PSUM tile, written
     as a single bf16 partial.
"""

import contextlib

import numpy as np

import concourse.bass as bass
import concourse.tile as tile
from concourse import bacc, mybir
from concourse import bass_utils

# Problem constants (hardcoded per contract)
B, N, DIM = 2, 2048, 1024
H, DH = 16, 64
INNER = H * DH
LN_EPS = 1e-5
ATTN_EPS = 1e-8
SCALE = DH ** -0.5

# Per-core constants
P = 128
T = N                 # tokens per core (one batch)
TT = T // P           # 16 token tiles of 128
NT4 = T // 512        # 4 token slabs of 512
KD = DIM // P         # 8 contraction tiles
HL = 4                # local heads per core
CQK = 2 * HL * DH     # 512 (q cols + k cols)
CV = HL * DH          # 256 (v cols)
GQK = CQK // P        # 4 col groups of 128
KT = T // P           # 16 key tiles of 128

# DVE fast-exp offload: number of 512-wide chunks (of 4) per pair-iter
# computed on the DVE via the Schraudolph bit trick in fp16.
EXP_DVE_CHUNKS = 2
FE_A = 1.4426950408889634 * 1024.0          # log2(e) * 2^10 (fp16 mantissa)
FE_B = 15.0 * 1024.0 - 58.0                 # fp16 exp bias - sawtooth adjust

f32 = mybir.dt.float32
fp16 = mybir.dt.float16
bf16 = mybir.dt.bfloat16
i16 = mybir.dt.int16
i32 = mybir.dt.int32
FT = mybir.ActivationFunctionType
ALU = mybir.AluOpType

import ml_dtypes
_BF16 = np.dtype(ml_dtypes.bfloat16)

_CACHE = {}


def _build(has_v0):
    nc = bacc.Bacc("TRN2", target_bir_lowering=False, debug=False)

    xt_d = nc.dram_tensor("xt", [DIM, T], fp16, kind="ExternalInput").ap()
    wqk_d = nc.dram_tensor("wqk", [DIM, CQK], fp16, kind="ExternalInput").ap()
    wva_d = nc.dram_tensor("wva", [DIM, CV + 1], fp16, kind="ExternalInput").ap()
    wout_d = nc.dram_tensor("wout", [2 * P, DIM], fp16, kind="ExternalInput").ap()
    nuqk_d = nc.dram_tensor("nuqk", [CQK], fp16, kind="ExternalInput").ap()
    nuv_d = nc.dram_tensor("nuv", [CV], fp16, kind="ExternalInput").ap()
    v0qk_d = nc.dram_tensor("v0qk", [CQK], f32, kind="ExternalInput").ap()
    v0v_d = nc.dram_tensor("v0v", [CV], f32, kind="ExternalInput").ap()
    outp_d = nc.dram_tensor("outp", [DIM, T], bf16, kind="ExternalOutput").ap()

    with tile.TileContext(nc) as tc, contextlib.ExitStack() as ctx:
        pers = ctx.enter_context(tc.tile_pool(name="pers", bufs=1))
        dram = ctx.enter_context(tc.tile_pool(name="dram", bufs=1, space="DRAM"))

        qkT = pers.tile([P, GQK, T], fp16)           # q/k transposed, stacked
        vaug = pers.tile([P, KT, HL, DH + 1], fp16)  # v + ones column
        outT = pers.tile([P, 2, T], fp16)            # attention out (transposed)
        wout_sb = pers.tile([P, 2, DIM], fp16)
        r_c = pers.tile([P, TT], f32)                # rsqrt(var+eps), col layout
        rFE_c = pers.tile([P, TT], f32)              # r * FE_A (fast-exp scale)
        mur_c = pers.tile([P, TT], f32)              # mu * r, col layout
        nmu_row = pers.tile([1, T], fp16)            # -mu, row layout
        dbc = pers.tile([P, 2, T // 2], f32)         # 1/denom broadcast (per qp)

        nmu_dram = dram.tile([1, T], fp16)
        r_dram = dram.tile([1, T], fp16)
        dnm_dram = dram.tile([2, T // 2], f32)
        rdn_dram = dram.tile([2, T // 2], f32)

        nc.vector.memset(vaug[:], 1.0)

        # ---------------- Phase A+B: stats + QKV projection ----------------
        with tc.tile_pool(name="pab", bufs=1) as pab, \
             tc.tile_pool(name="pabd", bufs=2) as pabd, \
             tc.tile_pool(name="pgen", bufs=4, space="PSUM") as pgen, \
             tc.tile_pool(name="pgv", bufs=3, space="PSUM") as pgv, \
             tc.tile_pool(name="pgs", bufs=1, space="PSUM") as pgs:

            xt_sb = pab.tile([P, KD, T], fp16)
            for t4 in range(NT4):
                tsl = slice(t4 * 512, (t4 + 1) * 512)
                for kt in range(KD):
                    nc.sync.dma_start(xt_sb[:, kt, tsl],
                                      xt_d[kt * P:(kt + 1) * P, tsl])
            wqk_sb = pab.tile([P, KD, CQK], fp16)
            nc.sync.dma_start(wqk_sb[:], wqk_d.rearrange("(o p) c -> p o c", p=P))
            wva_sb = pab.tile([P, KD, CV + 1], fp16)
            nc.sync.dma_start(wva_sb[:], wva_d.rearrange("(o p) c -> p o c", p=P))
            uqkr_sb = pab.tile([1, CQK], fp16)
            nc.sync.dma_start(uqkr_sb[:], nuqk_d[None, :])
            uv_bc = pab.tile([P, CV], fp16)
            nc.sync.dma_start(uv_bc[:], nuv_d[None, :].to_broadcast([P, CV]))
            if has_v0:
                v0qk_sb = pab.tile([P, GQK], f32)
                nc.sync.dma_start(v0qk_sb[:], v0qk_d.rearrange("(g p) -> p g", p=P))
                v0v_bc = pab.tile([P, CV], f32)
                nc.sync.dma_start(v0v_bc[:], v0v_d[None, :].to_broadcast([P, CV]))

            r_bc = pab.tile([P, T], fp16)
            sxc = pab.tile([P, TT], f32)
            sxxc = pab.tile([P, TT], f32)
            mu_cc = pab.tile([P, TT], f32)
            magic = pab.tile([P, TT], i32)
            nc.vector.memset(magic[:], 0x5F3759DF)
            ex2e = pab.tile([P, TT], f32)
            mu2 = pab.tile([P, TT], f32)
            ve = pab.tile([P, TT], f32)
            y0i = pab.tile([P, TT], i32)
            t0 = pab.tile([P, TT], f32)
            nmu16_c = pab.tile([P, TT], fp16)
            r16_c = pab.tile([P, TT], fp16)
            vtmp = pab.tile([P, HL, DH], f32)
            ones_col = pab.tile([P, 1], fp16)
            nc.vector.memset(ones_col[:], 1.0)

            bones = pab.tile([1, 1], fp16)
            nc.vector.memset(bones[:], 1.0)
            brow = pab.tile([1, 64], fp16)
            nc.vector.memset(brow[:], 1.0)
            warm_ps = pgen.tile([P, 512], f32, tag="qk", name="warm0")
            for _ in range(64):
                nc.tensor.matmul(warm_ps[64:65, 0:64], bones[:], brow[:],
                                 start=True, stop=True)

            # --- pass 1: v projection (+Sx rider) and Sxx, per token tile ---
            sxx_ps = pgs.tile([P, TT], f32, tag="sxx")
            for tt in range(TT):
                tsl = slice(tt * P, (tt + 1) * P)
                xsq = pabd.tile([P, KD, P], fp16, tag="xsq", name=f"xsq{tt}")
                nc.scalar.activation(xsq[:], xt_sb[:, :, tsl], FT.Square)
                psv = pgv.tile([P, CV + 1], f32, tag="v", name=f"v{tt}")
                for kt in range(KD):
                    nc.tensor.matmul(psv[:], xt_sb[:, kt, tsl], wva_sb[:, kt],
                                     start=(kt == 0), stop=(kt == KD - 1))
                for kt in range(KD):
                    nc.tensor.matmul(sxx_ps[:, tt:tt + 1], xsq[:, kt],
                                     ones_col[:],
                                     start=(kt == 0), stop=(kt == KD - 1))
                # v raw (unscaled) into vaug; Sx rider into column stats
                psv3 = psv[:, 0:CV].rearrange("p (h d) -> p h d", h=HL)
                nc.vector.tensor_copy(vaug[:, tt, :, 0:DH], psv3)
                nc.vector.tensor_copy(sxc[:, tt:tt + 1], psv[:, CV:CV + 1])
            nc.vector.tensor_copy(sxxc[:], sxx_ps[:])

            # --- rsqrt(var+eps) on the DVE, all-column layout ---
            nc.vector.tensor_scalar(ex2e[:], sxxc[:], 1.0 / DIM, LN_EPS,
                                    ALU.mult, ALU.add)
            nc.vector.tensor_scalar_mul(mu_cc[:], sxc[:], 1.0 / DIM)
            nc.vector.tensor_tensor(mu2[:], mu_cc[:], mu_cc[:], ALU.mult)
            nc.vector.scalar_tensor_tensor(ve[:], mu2[:], -1.0, ex2e[:],
                                           ALU.mult, ALU.add)
            nc.vector.tensor_scalar(y0i[:], ve[:].bitcast(i32), 1, None,
                                    ALU.arith_shift_right)
            nc.vector.tensor_tensor(y0i[:], magic[:], y0i[:], ALU.subtract)
            y = y0i.bitcast(f32)
            for _ in range(3):
                nc.vector.tensor_tensor(t0[:], y[:], y[:], ALU.mult)
                nc.vector.tensor_tensor(t0[:], t0[:], ve[:], ALU.mult)
                nc.vector.tensor_scalar(t0[:], t0[:], -0.5, 1.5,
                                        ALU.mult, ALU.add)
                nc.vector.tensor_tensor(y[:], y[:], t0[:], ALU.mult)
            nc.vector.tensor_copy(r_c[:], y[:])
            nc.vector.tensor_scalar_mul(rFE_c[:], r_c[:], FE_A)
            nc.vector.tensor_tensor(mur_c[:], mu_cc[:], r_c[:], ALU.mult)
            nc.vector.tensor_scalar_mul(nmu16_c[:], mu_cc[:], -1.0)
            nc.vector.tensor_copy(r16_c[:], r_c[:])

            # row layouts: -mu (rank-1 qk fix) and r broadcast (q scale)
            nc.sync.dma_start(nmu_dram[0, :].rearrange("(o p) -> p o", p=P),
                              nmu16_c[:])
            nc.sync.dma_start(nmu_row[:], nmu_dram[:])
            nc.sync.dma_start(r_dram[0, :].rearrange("(o p) -> p o", p=P),
                              r16_c[:])
            nc.sync.dma_start(r_bc[:], r_dram[0:1, :].to_broadcast([P, T]))

            # --- v fixup: vaug = r*(vraw - mu*uv) [+ v0] ---
            for tt in range(TT):
                nc.vector.tensor_scalar_mul(
                    vtmp.rearrange("p h d -> p (h d)"), uv_bc[:],
                    mur_c[:, tt:tt + 1])
                nc.vector.scalar_tensor_tensor(
                    vaug[:, tt, :, 0:DH], vaug[:, tt, :, 0:DH],
                    r_c[:, tt:tt + 1], vtmp[:], ALU.mult, ALU.subtract)
                if has_v0:
                    v03 = v0v_bc.rearrange("p (h d) -> p h d", h=HL)
                    nc.vector.tensor_tensor(vaug[:, tt, :, 0:DH],
                                            vaug[:, tt, :, 0:DH], v03, ALU.add)

            # --- pass 2: qk projection; k groups first (no r_bc dep).
            # Software-pipelined: main matmul groups run 4 units ahead of the
            # rank-1 fix + finish so the PE never stalls on the stats chain.
            units = [(g, t4) for g in (2, 3, 0, 1) for t4 in range(NT4)]
            ps_of = {}

            def qk_mains_group(g):
                pss = [pgen.tile([P, 512], f32, tag="qk", name=f"qk{g}_{t4}")
                       for t4 in range(NT4)]
                for kt in range(KD):
                    for t4 in range(NT4):
                        nc.tensor.matmul(pss[t4][:],
                                         wqk_sb[:, kt, g * P:(g + 1) * P],
                                         xt_sb[:, kt,
                                               t4 * 512:(t4 + 1) * 512],
                                         start=(kt == 0), stop=False)
                for t4 in range(NT4):
                    ps_of[(g, t4)] = pss[t4]

            def qk_finish(u):
                g, t4 = u
                tsl = slice(t4 * 512, (t4 + 1) * 512)
                ps = ps_of.pop(u)
                # rank-1 LayerNorm-mean correction: psum += u * (-mu)^T
                nc.tensor.matmul(ps[:], uqkr_sb[0:1, g * P:(g + 1) * P],
                                 nmu_row[0:1, tsl], start=False, stop=True)
                nc.vector.tensor_tensor(qkT[:, g, tsl], ps[:],
                                        r_bc[:, tsl], ALU.mult)
                if has_v0:
                    nc.vector.tensor_scalar_add(qkT[:, g, tsl],
                                                qkT[:, g, tsl],
                                                v0qk_sb[:, g:g + 1])

            for g in (2, 3, 0, 1):
                qk_mains_group(g)
                for t4 in range(NT4):
                    qk_finish((g, t4))

        # ---------------- Phase C: attention (head pairs) -------------------
        with tc.tile_pool(name="pat", bufs=2) as pat, \
             tc.tile_pool(name="pat1", bufs=1) as pat1, \
             tc.tile_pool(name="psc", bufs=2, space="PSUM") as psc, \
             tc.tile_pool(name="ppv", bufs=1, space="PSUM") as ppv:

            nc.sync.dma_start(wout_sb[:], wout_d.rearrange("(o p) c -> p o c", p=P))
            dn_row = [pat1.tile([1, T // 2], f32, name=f"dnr{i}")
                      for i in range(2)]
            wones = pat1.tile([1, 1], fp16)
            nc.vector.memset(wones[:], 1.0)
            wrow = pat1.tile([1, 64], fp16)
            nc.vector.memset(wrow[:], 1.0)
            dn2 = pat1.tile([P, 2, TT // 2], f32)
            rdn2 = pat1.tile([P, 2, TT // 2], f32)

            for hp in range(2):
                for qp in range(2):
                    qsl = slice(qp * 1024, (qp + 1) * 1024)
                    ps_o = {(half, sub): ppv.tile([DH + 1, 512], f32,
                                                  tag=f"pv{half}{sub}",
                                                  name=f"pv{hp}_{qp}_{half}{sub}")
                            for half in range(2) for sub in range(2)}
                    units = [(kt, sub) for kt in range(KT) for sub in range(2)]

                    def att_scores(u):
                        kt, sub = u
                        ksl = slice(kt * P, (kt + 1) * P)
                        qs = slice(qp * 1024 + sub * 512,
                                   qp * 1024 + (sub + 1) * 512)
                        ps_s = psc.tile([P, 2, 512], f32, tag="sc",
                                        name=f"sc{hp}_{qp}_{kt}_{sub}")
                        for half in range(2):  # head row-halves, row-tiled
                            rw = slice(64 * half, 64 * half + 64)
                            nc.tensor.matmul(ps_s[:, half],
                                             qkT[rw, 2 + hp, ksl],
                                             qkT[rw, hp, qs],
                                             start=True, stop=True)
                        return ps_s

                    def att_exp(ps_s, u, idx):
                        kt, sub = u
                        et = pat.tile([P, 2, 512], fp16, tag="exp",
                                      name=f"et{hp}_{qp}_{kt}_{sub}")
                        if idx % 2 == 0:
                            nc.scalar.activation(et[:], ps_s[:], FT.Exp)
                        else:
                            nc.vector.tensor_scalar(
                                et.bitcast(i16)[:], ps_s[:],
                                FE_A, FE_B, ALU.mult, ALU.add)
                        return et

                    def att_pv(et, u):
                        kt, sub = u
                        for half in range(2):
                            nc.tensor.matmul(ps_o[(half, sub)][:],
                                             vaug[:, kt, 2 * hp + half, :],
                                             et[:, half],
                                             start=(kt == 0), stop=(kt == KT - 1))

                    ps_prev = att_scores(units[0])
                    for i, u in enumerate(units):
                        et_i = att_exp(ps_prev, u, i)
                        if i + 1 < len(units):
                            ps_prev = att_scores(units[i + 1])
                        att_pv(et_i, u)
                    if hp == 1 and qp == 1:
                        # keep the PE HAM-warm through the final dn chain
                        wps = psc.tile([P, 2, 512], f32, tag="sc",
                                       name="warmtail")
                        for _ in range(40):
                            nc.tensor.matmul(wps[64:65, 0, 0:64],
                                             wones[:], wrow[:],
                                             start=True, stop=True)

                    # copy out (unnormalized) + denominator rows
                    for half in range(2):
                        rw = slice(64 * half, 64 * half + 64)
                        for sub in range(2):
                            qss = slice(qp * 1024 + sub * 512,
                                        qp * 1024 + (sub + 1) * 512)
                            nc.scalar.copy(outT[rw, hp, qss],
                                           ps_o[(half, sub)][0:DH])
                            nc.vector.tensor_copy(
                                dn_row[half][:, sub * 512:(sub + 1) * 512],
                                ps_o[(half, sub)][DH:DH + 1])
                        nc.sync.dma_start(dnm_dram[half:half + 1, :],
                                          dn_row[half][:])
                    # reciprocal in column layout, broadcast back
                    nc.sync.dma_start(
                        dn2[:], dnm_dram.rearrange("h (p o) -> p h o", p=P))
                    nc.vector.reciprocal(rdn2[:], dn2[:])
                    nc.sync.dma_start(
                        rdn_dram.rearrange("h (p o) -> p h o", p=P), rdn2[:])
                    for half in range(2):
                        rw = slice(64 * half, 64 * half + 64)
                        nc.sync.dma_start(
                            dbc[rw, qp, :],
                            rdn_dram[half:half + 1, :].to_broadcast([64, 1024]))
                        nc.vector.tensor_tensor(outT[rw, hp, qsl],
                                                outT[rw, hp, qsl],
                                                dbc[rw, qp, :], ALU.mult)

        # ---------------- Phase D: output projection ------------------------
        with tc.tile_pool(name="pdo", bufs=6) as pdo, \
             tc.tile_pool(name="pop", bufs=8, space="PSUM") as pop:
            for t4 in range(NT4):
                for oc in range(DIM // P):
                    tsl = slice(t4 * 512, (t4 + 1) * 512)
                    ps = pop.tile([P, 512], f32, tag="op", name=f"op{oc}_{t4}")
                    nc.tensor.matmul(ps[:], wout_sb[:, 0, oc * P:(oc + 1) * P],
                                     outT[:, 0, tsl], start=True, stop=False)
                    nc.tensor.matmul(ps[:], wout_sb[:, 1, oc * P:(oc + 1) * P],
                                     outT[:, 1, tsl], start=False, stop=True)
                    osb = pdo.tile([P, 512], bf16, tag="osb")
                    if (oc + t4) % 2 == 0:
                        nc.vector.tensor_copy(osb[:], ps[:])
                    else:
                        nc.scalar.copy(osb[:], ps[:])
                    nc.sync.dma_start(outp_d[oc * P:(oc + 1) * P, tsl], osb[:])

    nc.compile()
    return nc


def _prep_inputs(x, ln_gamma, ln_beta, w_qkv, w_out, b_out):
    """Host-side sharding/layout prep. Returns (in_maps, has_v0)."""
    x = np.asarray(x, dtype=np.float32)
    ln_gamma = np.asarray(ln_gamma, dtype=np.float32)
    ln_beta = np.asarray(ln_beta, dtype=np.float32)
    w_qkv = np.asarray(w_qkv, dtype=np.float32)
    w_out = np.asarray(w_out, dtype=np.float32)

    wsc = w_qkv.copy()
    wsc[:, :INNER] *= SCALE                      # fold attn scale into q
    wfold = ln_gamma[:, None] * wsc              # fold LN gamma
    u = wfold.sum(axis=0)                        # [3*INNER]
    v0 = ln_beta @ wsc                           # [3*INNER]
    has_v0 = bool(np.any(v0 != 0.0))

    wq, wk, wv_all = np.split(wfold, 3, axis=1)
    uq, uk, uv_all = np.split(u, 3)
    v0q, v0k, v0v_all = np.split(v0, 3)

    in_maps = []
    for c in range(8):
        b = c // 4
        hs = (c % 4) * HL * DH
        sl = slice(hs, hs + HL * DH)
        xb = x[b]                                           # [2048, 1024]
        wqk_loc = np.concatenate([wq[:, sl], wk[:, sl]], axis=1)  # [1024, 512]
        wva_loc = np.concatenate(
            [wv_all[:, sl], np.ones((DIM, 1), np.float32)], axis=1)
        in_maps.append({
            "xt": np.ascontiguousarray(xb.T).astype(np.float16),
            "wqk": np.ascontiguousarray(wqk_loc).astype(np.float16),
            "wva": np.ascontiguousarray(wva_loc).astype(np.float16),
            "wout": np.ascontiguousarray(w_out[sl, :]).astype(np.float16),
            "nuqk": np.concatenate([uq[sl], uk[sl]]).astype(np.float16),
            "nuv": uv_all[sl].astype(np.float16),
            "v0qk": np.concatenate([v0q[sl], v0k[sl]]).astype(np.float32),
            "v0v": v0v_all[sl].astype(np.float32),
        })
    return in_maps, has_v0


def run(x, ln_gamma, ln_beta, w_qkv, w_out, b_out, trace=False, trace_kwargs=None):
    in_maps, has_v0 = _prep_inputs(x, ln_gamma, ln_beta, w_qkv, w_out, b_out)
    key = ("nc", has_v0)
    if key not in _CACHE:
        _CACHE[key] = _build(has_v0)
    nc = _CACHE[key]
    kwargs = {}
    if trace:
        kwargs = dict(trace=True, trace_cores=[0],
                      stitch_traces=False, **(trace_kwargs or {}))
    res = bass_utils.run_bass_kernel_spmd(
        nc, in_maps, core_ids=list(range(8)), **kwargs)

    b_out = np.asarray(b_out, dtype=np.float32)
    out = np.zeros((B, N, DIM), dtype=np.float32)
    for b in range(B):
        acc = np.zeros((DIM, T), dtype=np.float32)
        for c in range(4 * b, 4 * b + 4):
            acc += np.asarray(res.results[c]["outp"]).astype(np.float32)
        out[b] = acc.T + b_out
    return out, res


def kernel(x, ln_gamma, ln_beta, w_qkv, w_out, b_out):
    out, _ = run(x, ln_gamma, ln_beta, w_qkv, w_out, b_out, trace=False)
    return out


# revision 22
# speedup vs baseline: 1.0282x; 1.0282x over previous
"""Trainium2 Bass kernel for nn_Attention (LN -> QKV -> softmax attn -> out proj).

Sharding: 8 cores; core c handles batch b=c//4 and heads [4*(c%4), 4*(c%4)+4).
Each core emits one bf16 partial [DIM, T]; the host sums 4 partials per batch,
transposes, and adds b_out.

Device pipeline per core (fp16 matmuls, fp32 PSUM):
  A) One pass over x^T tiles with x as the PE stationary:
     v = x^T-tile @ wv_aug (wv with a ones rider column -> Sx lands in column
     layout for free); Sxx via tiny N=1 matmuls with Square(x) stationary.
     rsqrt(var+eps) via bit-trick + Newton on the DVE, all in SBUF column
     layout (no DMA transposes). LN mean/gamma folds applied algebraically:
     qk via rank-1 (-mu u) PSUM accumulation, v via DVE outer-product fixup.
  B) QK projection: stationary = folded weights, moving = x^T -> q,k
     transposed [dh, t] with both heads of a stack split across partition
     halves. r_q folded into q via broadcast multiply; r_k rides the exp's
     per-partition scale.
  C) Attention in head pairs (one stack at a time): the two heads' score
     matmuls run concurrently in disjoint PE row groups (K=64 row tiling);
     one 2048-wide Exp per (pair, qp, kt) covers both heads; optionally a
     512-chunk is offloaded to the DVE as a bit-trick fast exp. P@V with a
     ones column appended to v accumulates the denominator in the same PSUM
     tile; per-qp reciprocal via DVE HW divide, normalize fused on copy-out.
  D) Output projection: both stacks accumulated into one PSUM tile, written
     as a single bf16 partial.
"""

import contextlib

import numpy as np

import concourse.bass as bass
import concourse.tile as tile
from concourse import bacc, mybir
from concourse import bass_utils

# Problem constants (hardcoded per contract)
B, N, DIM = 2, 2048, 1024
H, DH = 16, 64
INNER = H * DH
LN_EPS = 1e-5
ATTN_EPS = 1e-8
SCALE = DH ** -0.5

# Per-core constants
P = 128
T = N                 # tokens per core (one batch)
TT = T // P           # 16 token tiles of 128
NT4 = T // 512        # 4 token slabs of 512
KD = DIM // P         # 8 contraction tiles
HL = 4                # local heads per core
CQK = 2 * HL * DH     # 512 (q cols + k cols)
CV = HL * DH          # 256 (v cols)
GQK = CQK // P        # 4 col groups of 128
KT = T // P           # 16 key tiles of 128

# DVE fast-exp offload: number of 512-wide chunks (of 4) per pair-iter
# computed on the DVE via the Schraudolph bit trick in fp16.
EXP_DVE_CHUNKS = 2
FE_A = 1.4426950408889634 * 1024.0          # log2(e) * 2^10 (fp16 mantissa)
FE_B = 15.0 * 1024.0 - 58.0                 # fp16 exp bias - sawtooth adjust

f32 = mybir.dt.float32
fp16 = mybir.dt.float16
bf16 = mybir.dt.bfloat16
i16 = mybir.dt.int16
i32 = mybir.dt.int32
FT = mybir.ActivationFunctionType
ALU = mybir.AluOpType

import ml_dtypes
_BF16 = np.dtype(ml_dtypes.bfloat16)

_CACHE = {}


def _build(has_v0):
    nc = bacc.Bacc("TRN2", target_bir_lowering=False, debug=False)

    xt_d = nc.dram_tensor("xt", [DIM, T], fp16, kind="ExternalInput").ap()
    wqk_d = nc.dram_tensor("wqk", [DIM, CQK], fp16, kind="ExternalInput").ap()
    wva_d = nc.dram_tensor("wva", [DIM, CV + 1], fp16, kind="ExternalInput").ap()
    wout_d = nc.dram_tensor("wout", [2 * P, DIM], fp16, kind="ExternalInput").ap()
    nuqk_d = nc.dram_tensor("nuqk", [CQK], fp16, kind="ExternalInput").ap()
    nuv_d = nc.dram_tensor("nuv", [CV], fp16, kind="ExternalInput").ap()
    v0qk_d = nc.dram_tensor("v0qk", [CQK], f32, kind="ExternalInput").ap()
    v0v_d = nc.dram_tensor("v0v", [CV], f32, kind="ExternalInput").ap()
    outp_d = nc.dram_tensor("outp", [DIM, T], bf16, kind="ExternalOutput").ap()

    with tile.TileContext(nc) as tc, contextlib.ExitStack() as ctx:
        pers = ctx.enter_context(tc.tile_pool(name="pers", bufs=1))
        dram = ctx.enter_context(tc.tile_pool(name="dram", bufs=1, space="DRAM"))

        qkT = pers.tile([P, GQK, T], fp16)           # q/k transposed, stacked
        vaug = pers.tile([P, KT, HL, DH + 1], fp16)  # v + ones column
        outT = pers.tile([P, 2, T], fp16)            # attention out (transposed)
        wout_sb = pers.tile([P, 2, DIM], fp16)
        r_c = pers.tile([P, TT], f32)                # rsqrt(var+eps), col layout
        rFE_c = pers.tile([P, TT], f32)              # r * FE_A (fast-exp scale)
        mur_c = pers.tile([P, TT], f32)              # mu * r, col layout
        nmu_row = pers.tile([1, T], fp16)            # -mu, row layout
        dbc = pers.tile([P, T], f32)                 # 1/denom broadcast

        nmu_dram = dram.tile([1, T], fp16)
        r_dram = dram.tile([1, T], fp16)
        dnm_dram = dram.tile([2, T], f32)
        rdn_dram = dram.tile([2, T], f32)

        nc.vector.memset(vaug[:], 1.0)

        # ---------------- Phase A+B: stats + QKV projection ----------------
        with tc.tile_pool(name="pab", bufs=1) as pab, \
             tc.tile_pool(name="pabd", bufs=3) as pabd, \
             tc.tile_pool(name="pgen", bufs=4, space="PSUM") as pgen, \
             tc.tile_pool(name="pgv", bufs=3, space="PSUM") as pgv, \
             tc.tile_pool(name="pgs", bufs=1, space="PSUM") as pgs:

            xt_sb = pab.tile([P, KD, T], fp16)
            for t4 in range(NT4):
                tsl = slice(t4 * 512, (t4 + 1) * 512)
                for kt in range(KD):
                    nc.sync.dma_start(xt_sb[:, kt, tsl],
                                      xt_d[kt * P:(kt + 1) * P, tsl])
            wqk_sb = pab.tile([P, KD, CQK], fp16)
            nc.sync.dma_start(wqk_sb[:], wqk_d.rearrange("(o p) c -> p o c", p=P))
            wva_sb = pab.tile([P, KD, CV + 1], fp16)
            nc.sync.dma_start(wva_sb[:], wva_d.rearrange("(o p) c -> p o c", p=P))
            nc.sync.dma_start(wout_sb[:],
                              wout_d.rearrange("(o p) c -> p o c", p=P))
            uqkr_sb = pab.tile([1, CQK], fp16)
            nc.sync.dma_start(uqkr_sb[:], nuqk_d[None, :])
            uv_bc = pab.tile([P, CV], fp16)
            nc.sync.dma_start(uv_bc[:], nuv_d[None, :].to_broadcast([P, CV]))
            if has_v0:
                v0qk_sb = pab.tile([P, GQK], f32)
                nc.sync.dma_start(v0qk_sb[:], v0qk_d.rearrange("(g p) -> p g", p=P))
                v0v_bc = pab.tile([P, CV], f32)
                nc.sync.dma_start(v0v_bc[:], v0v_d[None, :].to_broadcast([P, CV]))

            r_bc = pab.tile([P, T], fp16)
            sxc = pab.tile([P, TT], f32)
            sxxc = pab.tile([P, TT], f32)
            mu_cc = pab.tile([P, TT], f32)
            magic = pab.tile([P, TT], i32)
            nc.vector.memset(magic[:], 0x5F3759DF)
            ex2e = pab.tile([P, TT], f32)
            mu2 = pab.tile([P, TT], f32)
            ve = pab.tile([P, TT], f32)
            y0i = pab.tile([P, TT], i32)
            t0 = pab.tile([P, TT], f32)
            nmu16_c = pab.tile([P, TT], fp16)
            r16_c = pab.tile([P, TT], fp16)
            vtmp = pab.tile([P, HL, DH], f32)
            ones_col = pab.tile([P, 1], fp16)
            nc.vector.memset(ones_col[:], 1.0)

            bones = pab.tile([1, 1], fp16)
            nc.vector.memset(bones[:], 1.0)
            brow = pab.tile([1, 64], fp16)
            nc.vector.memset(brow[:], 1.0)
            warm_ps = pgen.tile([P, 512], f32, tag="qk", name="warm0")
            for _ in range(64):
                nc.tensor.matmul(warm_ps[64:65, 0:64], bones[:], brow[:],
                                 start=True, stop=True)

            # --- pass 1: v projection (+Sx rider) and Sxx, per token tile ---
            sxx_ps = pgs.tile([P, TT], f32, tag="sxx")
            xsqs = {}

            def sxx_mms(tt):
                xsq = xsqs.pop(tt)
                for kt in range(KD):
                    nc.tensor.matmul(sxx_ps[:, tt:tt + 1], xsq[:, kt],
                                     ones_col[:],
                                     start=(kt == 0), stop=(kt == KD - 1))

            for tt in range(TT):
                tsl = slice(tt * P, (tt + 1) * P)
                xsq = xsqs[tt] = pabd.tile([P, KD, P], fp16, tag="xsq",
                                           name=f"xsq{tt}")
                nc.scalar.activation(xsq[:], xt_sb[:, :, tsl], FT.Square)
                psv = pgv.tile([P, CV + 1], f32, tag="v", name=f"v{tt}")
                for kt in range(KD):
                    nc.tensor.matmul(psv[:], xt_sb[:, kt, tsl], wva_sb[:, kt],
                                     start=(kt == 0), stop=(kt == KD - 1))
                if tt >= 2:
                    sxx_mms(tt - 2)
                # v raw (unscaled) into vaug; Sx rider into column stats
                psv3 = psv[:, 0:CV].rearrange("p (h d) -> p h d", h=HL)
                nc.vector.tensor_copy(vaug[:, tt, :, 0:DH], psv3)
                nc.vector.tensor_copy(sxc[:, tt:tt + 1], psv[:, CV:CV + 1])
            for tt in (TT - 2, TT - 1):
                sxx_mms(tt)
            nc.vector.tensor_copy(sxxc[:], sxx_ps[:])

            # --- rsqrt(var+eps) on the DVE, all-column layout ---
            nc.vector.tensor_scalar(ex2e[:], sxxc[:], 1.0 / DIM, LN_EPS,
                                    ALU.mult, ALU.add)
            nc.vector.tensor_scalar_mul(mu_cc[:], sxc[:], 1.0 / DIM)
            nc.vector.tensor_tensor(mu2[:], mu_cc[:], mu_cc[:], ALU.mult)
            nc.vector.scalar_tensor_tensor(ve[:], mu2[:], -1.0, ex2e[:],
                                           ALU.mult, ALU.add)
            nc.vector.tensor_scalar(y0i[:], ve[:].bitcast(i32), 1, None,
                                    ALU.arith_shift_right)
            nc.vector.tensor_tensor(y0i[:], magic[:], y0i[:], ALU.subtract)
            y = y0i.bitcast(f32)
            for _ in range(3):
                nc.vector.tensor_tensor(t0[:], y[:], y[:], ALU.mult)
                nc.vector.tensor_tensor(t0[:], t0[:], ve[:], ALU.mult)
                nc.vector.tensor_scalar(t0[:], t0[:], -0.5, 1.5,
                                        ALU.mult, ALU.add)
                nc.vector.tensor_tensor(y[:], y[:], t0[:], ALU.mult)
            nc.vector.tensor_copy(r_c[:], y[:])
            nc.vector.tensor_scalar_mul(rFE_c[:], r_c[:], FE_A)
            nc.vector.tensor_tensor(mur_c[:], mu_cc[:], r_c[:], ALU.mult)
            nc.vector.tensor_scalar_mul(nmu16_c[:], mu_cc[:], -1.0)
            nc.vector.tensor_copy(r16_c[:], r_c[:])

            # row layouts: -mu (rank-1 qk fix) and r broadcast (q scale)
            nc.sync.dma_start(nmu_dram[0, :].rearrange("(o p) -> p o", p=P),
                              nmu16_c[:])
            nc.sync.dma_start(nmu_row[:], nmu_dram[:])
            nc.sync.dma_start(r_dram[0, :].rearrange("(o p) -> p o", p=P),
                              r16_c[:])
            nc.sync.dma_start(r_bc[:], r_dram[0:1, :].to_broadcast([P, T]))

            # --- v fixup: vaug = r*(vraw - mu*uv) [+ v0] ---
            for tt in range(TT):
                nc.vector.tensor_scalar_mul(
                    vtmp.rearrange("p h d -> p (h d)"), uv_bc[:],
                    mur_c[:, tt:tt + 1])
                nc.vector.scalar_tensor_tensor(
                    vaug[:, tt, :, 0:DH], vaug[:, tt, :, 0:DH],
                    r_c[:, tt:tt + 1], vtmp[:], ALU.mult, ALU.subtract)
                if has_v0:
                    v03 = v0v_bc.rearrange("p (h d) -> p h d", h=HL)
                    nc.vector.tensor_tensor(vaug[:, tt, :, 0:DH],
                                            vaug[:, tt, :, 0:DH], v03, ALU.add)

            # --- pass 2: qk projection; k groups first (no r_bc dep).
            # Software-pipelined: main matmul groups run 4 units ahead of the
            # rank-1 fix + finish so the PE never stalls on the stats chain.
            units = [(g, t4) for g in (2, 3, 0, 1) for t4 in range(NT4)]
            ps_of = {}

            def qk_mains_group(g):
                pss = [pgen.tile([P, 512], f32, tag="qk", name=f"qk{g}_{t4}")
                       for t4 in range(NT4)]
                for kt in range(KD):
                    for t4 in range(NT4):
                        nc.tensor.matmul(pss[t4][:],
                                         wqk_sb[:, kt, g * P:(g + 1) * P],
                                         xt_sb[:, kt,
                                               t4 * 512:(t4 + 1) * 512],
                                         start=(kt == 0), stop=False)
                for t4 in range(NT4):
                    ps_of[(g, t4)] = pss[t4]

            def qk_finish(u):
                g, t4 = u
                tsl = slice(t4 * 512, (t4 + 1) * 512)
                ps = ps_of.pop(u)
                # rank-1 LayerNorm-mean correction: psum += u * (-mu)^T
                nc.tensor.matmul(ps[:], uqkr_sb[0:1, g * P:(g + 1) * P],
                                 nmu_row[0:1, tsl], start=False, stop=True)
                nc.vector.tensor_tensor(qkT[:, g, tsl], ps[:],
                                        r_bc[:, tsl], ALU.mult)
                if has_v0:
                    nc.vector.tensor_scalar_add(qkT[:, g, tsl],
                                                qkT[:, g, tsl],
                                                v0qk_sb[:, g:g + 1])

            for g in (2, 3, 0, 1):
                qk_mains_group(g)
                for t4 in range(NT4):
                    qk_finish((g, t4))

        # ---------------- Phase C: attention (head pairs) -------------------
        # 512-query blocks; whole-tile exp alternates ACT (exact) / DVE
        # (bit-trick); filler matmuls keep the PE HAM-warm; out-projection
        # interleaves into the hp=1 blocks as token ranges complete.
        with tc.tile_pool(name="pat", bufs=2) as pat, \
             tc.tile_pool(name="pat1", bufs=1) as pat1, \
             tc.tile_pool(name="pdo", bufs=4) as pdo, \
             tc.tile_pool(name="psc", bufs=2, space="PSUM") as psc, \
             tc.tile_pool(name="ppv", bufs=1, space="PSUM") as ppv, \
             tc.tile_pool(name="pwm", bufs=1, space="PSUM") as pwm, \
             tc.tile_pool(name="pop", bufs=1, space="PSUM") as pop:

            dn_row = [pat1.tile([1, 512], f32, name=f"dnr{i}") for i in range(2)]
            dn2 = pat1.tile([P, 2, 4], f32)
            rdn2 = pat1.tile([P, 2, 4], f32)
            wones = pat1.tile([1, 1], fp16)
            nc.vector.memset(wones[:], 1.0)
            wrow = pat1.tile([1, 64], fp16)
            nc.vector.memset(wrow[:], 1.0)
            wps = pwm.tile([P, 512], f32, tag="wps")

            NQB = 4                      # 512-query blocks per stack
            uidx = 0                     # global unit counter (engine parity)
            op_done = [0]

            def filler(n):
                for _ in range(n):
                    nc.tensor.matmul(wps[64:65, 0:64], wones[:], wrow[:],
                                     start=True, stop=True)

            def outproj_unit(tb, oc):
                tsl = slice(tb * 512, (tb + 1) * 512)
                ps = pop.tile([P, 512], f32, tag="op", name=f"op{tb}_{oc}")
                nc.tensor.matmul(ps[:], wout_sb[:, 0, oc * P:(oc + 1) * P],
                                 outT[:, 0, tsl], start=True, stop=False)
                nc.tensor.matmul(ps[:], wout_sb[:, 1, oc * P:(oc + 1) * P],
                                 outT[:, 1, tsl], start=False, stop=True)
                osb = pdo.tile([P, 512], bf16, tag="osb")
                if oc % 2 == 0:
                    nc.vector.tensor_copy(osb[:], ps[:])
                else:
                    nc.scalar.copy(osb[:], ps[:])
                nc.sync.dma_start(outp_d[oc * P:(oc + 1) * P, tsl], osb[:])
                op_done[0] += 1

            op_queue = [(tb, oc) for tb in range(NQB) for oc in range(DIM // P)]

            for hp in range(2):
                for qb in range(NQB):
                    qsl = slice(qb * 512, (qb + 1) * 512)
                    ps_o = {half: ppv.tile([P, 512], f32, tag=f"pv{half}",
                                           name=f"pv{hp}_{qb}_{half}")
                            for half in range(2)}

                    def att_scores(kt):
                        ksl = slice(kt * P, (kt + 1) * P)
                        ps_s = psc.tile([P, 2, 512], f32, tag="sc",
                                        name=f"sc{hp}_{qb}_{kt}")
                        for half in range(2):   # head row-halves, row-tiled
                            rw = slice(64 * half, 64 * half + 64)
                            nc.tensor.matmul(ps_s[:, half],
                                             qkT[rw, 2 + hp, ksl],
                                             qkT[rw, hp, qsl],
                                             start=True, stop=True)
                        return ps_s

                    def att_exp(ps_s, kt, idx):
                        et = pat.tile([P, 2, 512], fp16, tag="exp",
                                      name=f"et{hp}_{qb}_{kt}")
                        if idx % 2 == 0:
                            nc.scalar.activation(et[:], ps_s[:], FT.Exp)
                        else:
                            nc.vector.tensor_scalar(
                                et.bitcast(i16)[:], ps_s[:],
                                FE_A, FE_B, ALU.mult, ALU.add)
                        return et

                    def att_pv(et, kt):
                        for half in range(2):
                            nc.tensor.matmul(ps_o[half][0:DH + 1],
                                             vaug[:, kt, 2 * hp + half, :],
                                             et[:, half],
                                             start=(kt == 0), stop=(kt == KT - 1))

                    ps_prev = att_scores(0)
                    for kt in range(KT):
                        et_i = att_exp(ps_prev, kt, uidx)
                        uidx += 1
                        if kt + 1 < KT:
                            ps_prev = att_scores(kt + 1)
                        if hp == 1 and qb >= 1 and kt % 2 == 0 and \
                                op_done[0] < qb * DIM // P:
                            outproj_unit(*op_queue[op_done[0]])
                        else:
                            filler(3)
                        att_pv(et_i, kt)

                    # copy out (unnormalized) + denominator rows
                    for half in range(2):
                        rw = slice(64 * half, 64 * half + 64)
                        nc.scalar.copy(outT[rw, hp, qsl], ps_o[half][0:DH])
                        nc.vector.tensor_copy(dn_row[half][:],
                                              ps_o[half][DH:DH + 1])
                        nc.sync.dma_start(
                            dnm_dram[half:half + 1, qsl], dn_row[half][:])
                    # reciprocal in column layout, broadcast back
                    nc.sync.dma_start(
                        dn2[:], dnm_dram[:, qsl].rearrange(
                            "h (p o) -> p h o", p=P))
                    nc.vector.reciprocal(rdn2[:], dn2[:])
                    nc.sync.dma_start(
                        rdn_dram[:, qsl].rearrange("h (p o) -> p h o", p=P),
                        rdn2[:])
                    for half in range(2):
                        rw = slice(64 * half, 64 * half + 64)
                        nc.sync.dma_start(
                            dbc[rw, qsl],
                            rdn_dram[half:half + 1, qsl].to_broadcast(
                                [64, 512]))
                        nc.vector.tensor_tensor(outT[rw, hp, qsl],
                                                outT[rw, hp, qsl],
                                                dbc[rw, qsl], ALU.mult)
                    if hp == 1 and qb == NQB - 1:
                        filler(40)       # stay warm through the last dn chain

            # ------------ Phase D: remaining output projection --------------
            while op_done[0] < len(op_queue):
                outproj_unit(*op_queue[op_done[0]])

    nc.compile()
    return nc


def _prep_inputs(x, ln_gamma, ln_beta, w_qkv, w_out, b_out):
    """Host-side sharding/layout prep. Returns (in_maps, has_v0)."""
    x = np.asarray(x, dtype=np.float32)
    ln_gamma = np.asarray(ln_gamma, dtype=np.float32)
    ln_beta = np.asarray(ln_beta, dtype=np.float32)
    w_qkv = np.asarray(w_qkv, dtype=np.float32)
    w_out = np.asarray(w_out, dtype=np.float32)

    wsc = w_qkv.copy()
    wsc[:, :INNER] *= SCALE                      # fold attn scale into q
    wfold = ln_gamma[:, None] * wsc              # fold LN gamma
    u = wfold.sum(axis=0)                        # [3*INNER]
    v0 = ln_beta @ wsc                           # [3*INNER]
    has_v0 = bool(np.any(v0 != 0.0))

    wq, wk, wv_all = np.split(wfold, 3, axis=1)
    uq, uk, uv_all = np.split(u, 3)
    v0q, v0k, v0v_all = np.split(v0, 3)

    in_maps = []
    for c in range(8):
        b = c // 4
        hs = (c % 4) * HL * DH
        sl = slice(hs, hs + HL * DH)
        xb = x[b]                                           # [2048, 1024]
        wqk_loc = np.concatenate([wq[:, sl], wk[:, sl]], axis=1)  # [1024, 512]
        wva_loc = np.concatenate(
            [wv_all[:, sl], np.ones((DIM, 1), np.float32)], axis=1)
        in_maps.append({
            "xt": np.ascontiguousarray(xb.T).astype(np.float16),
            "wqk": np.ascontiguousarray(wqk_loc).astype(np.float16),
            "wva": np.ascontiguousarray(wva_loc).astype(np.float16),
            "wout": np.ascontiguousarray(w_out[sl, :]).astype(np.float16),
            "nuqk": np.concatenate([uq[sl], uk[sl]]).astype(np.float16),
            "nuv": uv_all[sl].astype(np.float16),
            "v0qk": np.concatenate([v0q[sl], v0k[sl]]).astype(np.float32),
            "v0v": v0v_all[sl].astype(np.float32),
        })
    return in_maps, has_v0


def run(x, ln_gamma, ln_beta, w_qkv, w_out, b_out, trace=False, trace_kwargs=None):
    in_maps, has_v0 = _prep_inputs(x, ln_gamma, ln_beta, w_qkv, w_out, b_out)
    key = ("nc", has_v0)
    if key not in _CACHE:
        _CACHE[key] = _build(has_v0)
    nc = _CACHE[key]
    kwargs = {}
    if trace:
        kwargs = dict(trace=True, trace_cores=[0],
                      stitch_traces=False, **(trace_kwargs or {}))
    res = bass_utils.run_bass_kernel_spmd(
        nc, in_maps, core_ids=list(range(8)), **kwargs)

    b_out = np.asarray(b_out, dtype=np.float32)
    out = np.zeros((B, N, DIM), dtype=np.float32)
    for b in range(B):
        acc = np.zeros((DIM, T), dtype=np.float32)
        for c in range(4 * b, 4 * b + 4):
            acc += np.asarray(res.results[c]["outp"]).astype(np.float32)
        out[b] = acc.T + b_out
    return out, res


def kernel(x, ln_gamma, ln_beta, w_qkv, w_out, b_out):
    out, _ = run(x, ln_gamma, ln_beta, w_qkv, w_out, b_out, trace=False)
    return out
